# revision 9
# baseline (speedup 1.0000x reference)
"""Trainium2 Bass kernel for DeepSeek-V3-style block-sparse MoE MLP.

Strategy v4 (host-routed sparse dispatch, token-stationary matmuls):
  - Routing (group-limited top-k) is computed EXACTLY on the host in
    numpy (selection margins ~1e-4 >> f32 noise, so it matches the jax
    reference), and each core's 4 experts get their assigned tokens
    gathered into C=128 zero-padded slots.
  - Matmuls are token-stationary: the gathered-token tile [h', C] is the
    (reused) stationary operand and the fp16 weights stream through as
    512-wide moving operands. This amortizes the PE weight-load overhead
    that dominates at 128-wide free dims (measured ~95ns/matmul v2 vs
    ~55ns of useful stream), cutting PE time ~37.5us -> ~27us so it
    hides under the ~33us weight-DMA stream.
  - Activations a = silu(g) * (u * w) are computed in [slot, i] layout
    (routing weight w folded in via a per-partition tensor_scalar), then
    flipped to [i, slot] with 4 PE-transposes per expert for the
    down-proj, which emits the weighted partial output [slot, h].
  - The host scatter-adds the per-expert [cnt, H] panels into [T, H].
  - DMA: one need-ordered sync ring for inputs (per-half-expert gu
    chunks; the last expert's wd split so the final bytes gate only the
    last down-proj pass); outputs drain on the scalar-engine ring.
"""
import sys
sys.path.insert(0, '/opt/trn_rl_repo')
import numpy as np
import concourse.mybir as mybir
import concourse.tile as tile
from concourse import bass
from concourse.bass_utils import run_bass_kernel_spmd

T, H, I, E = 256, 1024, 512, 32
N_CORES = 8
E_LOC = E // N_CORES            # 4 experts per core
N_GROUP, GSZ = 8, 4
TOP_K = 8
TOPK_GROUP = 4
ROUTED_SCALING_FACTOR = 2.5
P = 128
NHC = H // P                    # 8 h-chunks (contraction for up/gate)
NIB = I // P                    # 4 i-blocks
dt = mybir.dt
F32, F16 = dt.float32, dt.float16
Act = mybir.ActivationFunctionType
Alu = mybir.AluOpType

_CACHE = {}


def _build(C):
    nc = bass.Bass('TRN2')
    xg_d = nc.dram_tensor('xg', [P, E_LOC * NHC * C], F16, kind='ExternalInput')
    wvec_d = nc.dram_tensor('wvec', [P, E_LOC], F32, kind='ExternalInput')
    ident_d = nc.dram_tensor('ident', [P, C], F16, kind='ExternalInput')
    # gu: [p=h', e, c, {g,u}, i(512)]   wd: [p=i', e, ic, h(1024)]
    gu_d = nc.dram_tensor('gu', [P, E_LOC * NHC * 2 * I], F16,
                          kind='ExternalInput')
    wd_d = nc.dram_tensor('wd', [P, E_LOC * NIB * H], F16, kind='ExternalInput')
    out_d = nc.dram_tensor('out', [E_LOC * P, H], F16, kind='ExternalOutput')

    GUSEG = NHC * 2 * I             # per-expert elems/partition in gu
    GUCH = GUSEG // 4               # per-(expert, 2 c-chunks)
    WDSEG = NIB * H

    with tile.TileContext(nc) as tc:
        with tc.tile_pool(name='consts', bufs=1) as consts, \
             tc.tile_pool(name='wpool', bufs=1) as wpool, \
             tc.tile_pool(name='actp', bufs=2) as actp, \
             tc.tile_pool(name='atp', bufs=2) as atp, \
             tc.tile_pool(name='outp', bufs=1) as outp, \
             tc.tile_pool(name='ps', bufs=1, space='PSUM') as ps, \
             tc.tile_pool(name='psy', bufs=1, space='PSUM') as psy:

            # ---------- PE warmup (ramps the PE clock during DMA head) ----
            scratch = consts.tile([P, 512], F16)
            nc.vector.memset(scratch, 0.0)
            pwarm = ps.tile([P, 512], F32, name='pwarm', tag='ps_pg', bufs=2)
            N_WARM = 8
            for i in range(N_WARM):
                nc.tensor.matmul(pwarm, lhsT=scratch[:, 0:128],
                                 rhs=scratch, start=(i == 0),
                                 stop=(i == N_WARM - 1))

            # ---------- SBUF tiles ----------
            xg_sb = consts.tile([P, E_LOC, NHC, C], F16)
            wvec_sb = consts.tile([P, E_LOC], F32)
            ident_sb = consts.tile([P, C], F16)
            wgu_sb, wd_sb = [], []
            for e in range(E_LOC):
                wgu_sb.append(wpool.tile([P, NHC, 2, I], F16,
                                         name=f'wgu{e}', tag=f'wgu{e}'))
                wd_sb.append(wpool.tile([P, NIB, H], F16,
                                        name=f'wd{e}', tag=f'wd{e}'))

            def dma_gu(e, q):
                # quarter q covers c-chunks 2q, 2q+1
                nc.sync.dma_start(
                    wgu_sb[e][:, 2 * q:2 * q + 2].rearrange(
                        "p c t i -> p (c t i)"),
                    gu_d[:, e * GUSEG + q * GUCH:e * GUSEG + (q + 1) * GUCH])

            def dma_wd(e, half=None):
                if half is None:
                    nc.sync.dma_start(
                        wd_sb[e].rearrange("p b h -> p (b h)"),
                        wd_d[:, e * WDSEG:(e + 1) * WDSEG])
                else:
                    nc.sync.dma_start(
                        wd_sb[e][:, 2 * half:2 * half + 2].rearrange(
                            "p b h -> p (b h)"),
                        wd_d[:, e * WDSEG + half * (WDSEG // 2):
                             e * WDSEG + (half + 1) * (WDSEG // 2)])

            # ---------- input DMAs: sync ring, need-order ----------
            nc.sync.dma_start(xg_sb[:, 0].rearrange("p c t -> p (c t)"),
                              xg_d[:, 0:NHC * C])
            for q in range(4):
                dma_gu(0, q)
            nc.sync.dma_start(wvec_sb, wvec_d[:, :])
            nc.sync.dma_start(ident_sb, ident_d[:, :])
            nc.sync.dma_start(
                xg_sb[:, 1:E_LOC].rearrange("p e c t -> p (e c t)"),
                xg_d[:, NHC * C:E_LOC * NHC * C])
            dma_wd(0)

            # ---------- per-expert compute ----------
            att = {}

            def emit_ug(e):
                pg = ps.tile([C, I], F32, name=f'pg{e}', tag='ps_pg', bufs=2)
                pu = ps.tile([C, I], F32, name=f'pu{e}', tag='ps_pu', bufs=2)
                for c in range(NHC):
                    nc.tensor.matmul(pg, lhsT=xg_sb[:, e, c, :],
                                     rhs=wgu_sb[e][:, c, 0, :],
                                     start=(c == 0), stop=(c == NHC - 1))
                    nc.tensor.matmul(pu, lhsT=xg_sb[:, e, c, :],
                                     rhs=wgu_sb[e][:, c, 1, :],
                                     start=(c == 0), stop=(c == NHC - 1))
                # puw = u * w (DVE, per-partition scalar) in parallel
                # with silu (Act engine)
                puw = actp.tile([C, I], F32, name=f'puw{e}', tag='puw')
                nc.vector.tensor_scalar(puw, pu, wvec_sb[:, e:e + 1], None,
                                        op0=Alu.mult)
                sg = actp.tile([C, I], F32, name=f'sg{e}', tag='sg')
                nc.scalar.activation(sg, pg, Act.Silu)
                at = actp.tile([C, I], F16, name=f'at{e}', tag='at')
                nc.vector.tensor_mul(at, sg, puw)
                return at

            def emit_tr(e, at):
                # flip a [slot, i] -> [i', slot] per i-block via PE transpose
                pt = ps.tile([P, NIB, C], F16, name=f'pt{e}', tag='ps_pt',
                             bufs=1)
                for ic in range(NIB):
                    nc.tensor.transpose(pt[:, ic, :],
                                        at[:, ic * P:(ic + 1) * P], ident_sb)
                a = atp.tile([P, NIB, C], F16, name=f'att{e}', tag='att',
                             bufs=2)
                nc.vector.tensor_copy(a, pt)
                att[e] = a

            def emit_down(e):
                yb = [psy.tile([C, 512], F32, name=f'y{e}_{half}',
                               tag=f'ps_y{half}', bufs=1) for half in range(2)]
                for ic in range(NIB):
                    for half in range(2):
                        nc.tensor.matmul(
                            yb[half],
                            lhsT=att[e][:, ic, :],
                            rhs=wd_sb[e][:, ic, half * 512:(half + 1) * 512],
                            start=(ic == 0), stop=(ic == NIB - 1))
                osb = outp.tile([C, H], F16, name=f'osb{e}', tag=f'osb{e}')
                nc.vector.tensor_copy(osb[:, 0:512], yb[0])
                nc.vector.tensor_copy(osb[:, 512:1024], yb[1])
                # outputs drain on the (otherwise idle) gpsimd ring so
                # neither the weight stream nor the act/vector pipelines
                # queue behind them
                nc.gpsimd.dma_start(out_d[e * P:(e + 1) * P, :], osb)

            # software pipeline; sync-ring emission order == transfer order
            at0 = emit_ug(0)
            for q in range(4):
                dma_gu(1, q)
            emit_tr(0, at0)
            emit_down(0)
            dma_wd(1)
            at1 = emit_ug(1)
            for q in range(4):
                dma_gu(2, q)
            emit_tr(1, at1)
            emit_down(1)
            dma_wd(2)
            at2 = emit_ug(2)
            for q in range(4):
                dma_gu(3, q)
            emit_tr(2, at2)
            at3 = emit_ug(3)
            dma_wd(3, 0)
            dma_wd(3, 1)
            emit_down(2)
            emit_tr(3, at3)
            emit_down(3)

    _spill_excess_waits(nc)
    return nc


def _spill_excess_waits(nc, max_waits=1):
    """walrus codegen in this container accepts at most one semaphore wait
    per engine instruction; move extra waits onto preceding same-engine NOPs
    (engine queues are in-order, so this preserves the synchronization)."""
    f = nc.m.functions[0]
    for b in f.blocks:
        new_insts = []
        for inst in b.instructions:
            si = inst.sync_info
            if si is not None and si.on_wait is not None \
                    and len(si.on_wait) > max_waits:
                waits = list(si.on_wait)
                keep = waits[-max_waits:]
                extra = waits[:-max_waits]
                for k, w in enumerate(extra):
                    nop = mybir.InstNoOp(
                        name=f"{inst.name}-wspill{k}",
                        sync_info=mybir.SyncInfo(on_wait=[w], on_update=[]),
                        bass_nofuse=True,
                        engine=inst.engine,
                    )
                    new_insts.append(nop)
                inst.sync_info = mybir.SyncInfo(
                    on_wait=keep, on_update=list(si.on_update or []))
            new_insts.append(inst)
        b.instructions = new_insts


# ---------------- host-side routing (exact numpy replica) ----------------

def _topk_np(a, k):
    # ties broken by lower index, like jax.lax.top_k
    idx = np.argsort(-a, axis=-1, kind='stable')[..., :k]
    return np.take_along_axis(a, idx, axis=-1), idx


def _route_ds3_np(x, gate_w, e_score_bias):
    logits = x.astype(np.float32) @ gate_w.astype(np.float32)
    scores = 1.0 / (1.0 + np.exp(-logits))
    s4c = scores + e_score_bias[None, :].astype(np.float32)
    gsz = E // N_GROUP
    grouped = s4c.reshape(-1, N_GROUP, gsz)
    g2, _ = _topk_np(grouped, 2)
    _, group_idx = _topk_np(g2.sum(-1), TOPK_GROUP)
    group_mask = np.zeros((x.shape[0], N_GROUP), np.float32)
    np.put_along_axis(group_mask, group_idx, 1.0, axis=1)
    masked = np.where(np.repeat(group_mask, gsz, axis=-1) > 0, s4c, 0.0)
    _, topk_idx = _topk_np(masked, TOP_K)
    topk_w = np.take_along_axis(scores, topk_idx, axis=1)
    topk_w = topk_w / (topk_w.sum(-1, keepdims=True) + 1e-20)
    return topk_idx, topk_w * ROUTED_SCALING_FACTOR


def kernel(x, gate_w, e_score_bias, Wg, Wu, Wd):
    f16 = np.float16
    x = np.asarray(x, dtype=np.float32)
    topk_idx, topk_w = _route_ds3_np(
        x, np.asarray(gate_w), np.asarray(e_score_bias))

    toks, ws = [], []
    for e in range(E):
        te, je = np.nonzero(topk_idx == e)
        toks.append(te)
        ws.append(topk_w[te, je].astype(np.float32))
    max_cnt = max(len(t) for t in toks)
    assert max_cnt <= P, f"expert capacity exceeded: {max_cnt} > {P}"
    C = P

    if _CACHE.get('C') != C:
        _CACHE['C'] = C
        _CACHE['nc'] = _build(C)
    nc = _CACHE['nc']

    # x^T in partition-major layout [p=h', c, t]
    xTp = np.ascontiguousarray(
        x.T.reshape(NHC, P, T).transpose(1, 0, 2)).astype(f16)

    Wg_ = np.asarray(Wg).astype(f16)
    Wu_ = np.asarray(Wu).astype(f16)
    Wd_ = np.asarray(Wd).astype(f16)
    # gu host layout: [e][p=h', c, {g,u}, i]
    gu_all = np.empty((E, P, NHC, 2, I), f16)
    gu_all[:, :, :, 0] = Wg_.reshape(E, NHC, P, I).transpose(0, 2, 1, 3)
    gu_all[:, :, :, 1] = Wu_.reshape(E, NHC, P, I).transpose(0, 2, 1, 3)
    # wd host layout: [e][p=i', ic, h]
    wd_all = Wd_.reshape(E, NIB, P, H).transpose(0, 2, 1, 3)

    ident = np.eye(P, C, dtype=f16)

    in_maps = []
    for c in range(N_CORES):
        xg = np.zeros((P, E_LOC, NHC, C), f16)
        wvec = np.zeros((P, E_LOC), np.float32)
        for j in range(E_LOC):
            e = c * E_LOC + j
            tl = toks[e]
            xg[:, j, :, :len(tl)] = xTp[:, :, tl]
            wvec[:len(tl), j] = ws[e]
        esl = slice(c * E_LOC, (c + 1) * E_LOC)
        in_maps.append({
            'xg': np.ascontiguousarray(xg).reshape(P, -1),
            'wvec': wvec,
            'ident': ident,
            'gu': np.ascontiguousarray(
                gu_all[esl].transpose(1, 0, 2, 3, 4)).reshape(P, -1),
            'wd': np.ascontiguousarray(
                wd_all[esl].transpose(1, 0, 2, 3)).reshape(P, -1),
        })

    _CACHE['in_maps'] = in_maps
    res = run_bass_kernel_spmd(nc, in_maps, core_ids=list(range(N_CORES)))

    out = np.zeros((T, H), dtype=np.float32)
    for c in range(N_CORES):
        arr = res.results[c]['out'].astype(np.float32)  # [E_LOC*P, H]
        for j in range(E_LOC):
            e = c * E_LOC + j
            tl = toks[e]
            if len(tl):
                out[tl] += arr[j * P:j * P + len(tl), :]
    return out


def run_traced(**kwargs):
    """Re-run the last kernel invocation with NTFF tracing enabled."""
    return run_bass_kernel_spmd(_CACHE['nc'], _CACHE['in_maps'],
                                core_ids=list(range(N_CORES)), trace=True,
                                **kwargs)
